# revision 9
# baseline (speedup 1.0000x reference)
"""ArgmaxIOU kernel for 8 Trainium2 NeuronCores.

Data-parallel over batch: core i processes sample i (shapes hardcoded:
B=8, C=21, H=W=512). Raw Bass (explicit engines + semaphores).

Per 128x(21x256) pixel tile, per core:
  gpsimd: DMA f32 [P, C, T] tiles from HBM (double-buffered)
  ACT:    convert f32 -> bf16 into layout [P, TB, C, G] (G=4 pixel columns
          interleaved innermost so each matmul block is contiguous)
  DVE:    pairwise-max tree over the 21 classes (bf16 2x mode), then
          is_equal against the broadcast max -> one-hot argmax mask,
          written in place over the bf16 data
  PE:     matmul eqt^T @ eqp per G-block, accumulating a packed [84, 84]
          confusion matrix in PSUM: conf[c,c'] = sum_g out[4c+g, 4c'+g]

Host: gather the 8 packed matrices, fold G, compute mean IoU (trivial).
The bf16 argmax quantization changes the score by ~1e-4 relative (ties
in bf16 are rare and wash out of the large confusion counts).
"""

import sys

import numpy as np

for p in ("/opt/trn_rl_repo",):
    if p not in sys.path:
        sys.path.insert(0, p)

from concourse import bass, mybir
from concourse.bass_utils import run_bass_kernel_spmd

B = 8
C = 21
HW = 512 * 512
T = 256                  # pixels per partition per tile
P = 128                  # partitions
J = HW // (P * T)        # number of pixel tiles (8)
G = 4                    # t-columns packed per matmul (4*21=84 <= 128)
TB = T // G              # matmul blocks per tile (64)
M = G * C                # 84

F32 = mybir.dt.float32
BF16 = mybir.dt.bfloat16


def _tree_and_eq(v, data, scr, mout):
    """DVE: max over 21 class slices of data [P, TB, C, G] -> mout [P, TB, G],
    then in-place is_equal(data, broadcast max) -> one-hot mask."""
    mx = mybir.AluOpType.max
    v.tensor_tensor(scr[:], data[:, :, 0:10, :], data[:, :, 10:20, :], mx)
    v.tensor_tensor(scr[:, :, 0:5, :], scr[:, :, 0:5, :], scr[:, :, 5:10, :], mx)
    v.tensor_tensor(scr[:, :, 0:2, :], scr[:, :, 0:2, :], scr[:, :, 2:4, :], mx)
    v.tensor_tensor(scr[:, :, 0:1, :], scr[:, :, 0:1, :], scr[:, :, 1:2, :], mx)
    v.tensor_tensor(scr[:, :, 0:1, :], scr[:, :, 0:1, :], scr[:, :, 4:5, :], mx)
    v.tensor_tensor(mout[:].unsqueeze(2), scr[:, :, 0:1, :],
                    data[:, :, 20:21, :], mx)
    return v.tensor_tensor(
        data[:], data[:], mout[:].unsqueeze(2).broadcast_to((P, TB, C, G)),
        mybir.AluOpType.is_equal)


def build():
    nc = bass.Bass()
    pred = nc.declare_dram_parameter("prediction", [C, HW], F32, isOutput=False)
    targ = nc.declare_dram_parameter("target", [C, HW], F32, isOutput=False)
    out = nc.declare_dram_parameter("out", [M, M], F32, isOutput=True)

    predv = pred[:].rearrange("c (j p t) -> j p c t", j=J, p=P, t=T)
    targv = targ[:].rearrange("c (j p t) -> j p c t", j=J, p=P, t=T)

    cp = mybir.ActivationFunctionType.Copy

    with (
        nc.sbuf_tensor("ft", [P, 2, C * T], F32) as ft,
        nc.sbuf_tensor("fp", [P, 2, C * T], F32) as fp,
        nc.sbuf_tensor("bt", [P, 2, TB, C, G], BF16) as bt,
        nc.sbuf_tensor("bp", [P, 2, TB, C, G], BF16) as bp,
        nc.sbuf_tensor("st", [P, TB, 10, G], BF16) as st,
        nc.sbuf_tensor("sp", [P, TB, 10, G], BF16) as sp,
        nc.sbuf_tensor("mt", [P, TB, G], BF16) as mt,
        nc.sbuf_tensor("mp", [P, TB, G], BF16) as mp,
        nc.sbuf_tensor("osb", [M, M], F32) as osb,
        nc.psum_tensor("conf", [M, M], F32) as conf,
        nc.semaphore("dml") as dml,
        nc.semaphore("cvt") as cvt,
        nc.semaphore("dve") as dve,
        nc.semaphore("mm") as mm,
        nc.semaphore("fin") as fin,
        nc.Block() as block,
    ):

        @block.gpsimd
        def _(g):
            for j in range(J):
                s = j % 2
                if j >= 2:
                    g.wait_ge(cvt, 2 * (j - 2) + 1)  # convT_{j-2} read ft slot
                g.dma_start(
                    out=ft[:, s].rearrange("p (c t) -> p c t", c=C),
                    in_=targv[j]).then_inc(dml, 16)
                if j >= 2:
                    g.wait_ge(cvt, 2 * (j - 2) + 2)
                g.dma_start(
                    out=fp[:, s].rearrange("p (c t) -> p c t", c=C),
                    in_=predv[j]).then_inc(dml, 16)
            g.wait_ge(dve, 2 * J + 1)                # osb written
            g.dma_start(out=out[:], in_=osb[:]).then_inc(fin, 16)
            g.wait_ge(fin, 16)

        @block.scalar
        def _(sc):
            for j in range(J):
                s = j % 2
                if j >= 2:
                    sc.wait_ge(mm, j - 1)            # PE done with bt/bp slot
                sc.wait_ge(dml, 32 * j + 16)
                sc.activation(
                    bt[:, s].rearrange("p tb c g -> p c tb g"),
                    ft[:, s].rearrange("p (c tb g) -> p c tb g", c=C, g=G),
                    cp).then_inc(cvt, 1)
                sc.wait_ge(dml, 32 * j + 32)
                sc.activation(
                    bp[:, s].rearrange("p tb c g -> p c tb g"),
                    fp[:, s].rearrange("p (c tb g) -> p c tb g", c=C, g=G),
                    cp).then_inc(cvt, 1)

        @block.vector
        def _(v):
            for j in range(J):
                s = j % 2
                v.wait_ge(cvt, 2 * j + 1)
                _tree_and_eq(v, bt[:, s], st, mt).then_inc(dve, 1)
                v.wait_ge(cvt, 2 * j + 2)
                _tree_and_eq(v, bp[:, s], sp, mp).then_inc(dve, 1)
            v.wait_ge(mm, J)
            v.tensor_copy(osb[:], conf[:]).then_inc(dve, 1)

        @block.tensor
        def _(te):
            for j in range(J):
                s = j % 2
                te.wait_ge(dve, 2 * j + 2)
                for tb in range(TB):
                    inst = te.matmul(
                        conf[:],
                        bt[:, s, tb].rearrange("p c g -> p (c g)"),
                        bp[:, s, tb].rearrange("p c g -> p (c g)"),
                        start=(j == 0 and tb == 0),
                        stop=(j == J - 1 and tb == TB - 1))
                    if tb == TB - 1:
                        inst.then_inc(mm, 1)

    return nc


def _score_from_packed(packed):
    """packed: [84, 84] f32 -> per-sample mean IoU (float64)."""
    x = packed.astype(np.float64).reshape(C, G, C, G)
    conf = np.einsum("igjg->ij", x)
    TP = np.diag(conf).copy()
    FN = conf.sum(axis=1) - TP
    FP = conf.sum(axis=0) - TP
    valid = TP > 0
    denom = TP + FN + FP
    iou = np.where(valid, TP / np.where(valid, denom, 1.0), 0.0)
    n_valid = max(float(valid.sum()), 1.0)
    return iou.sum() / n_valid


_NC_CACHE = {}


def _get_nc():
    if "nc" not in _NC_CACHE:
        _NC_CACHE["nc"] = build()
    return _NC_CACHE["nc"]


def run(prediction, target, trace=False):
    in_maps = []
    for i in range(B):
        in_maps.append({
            "prediction": np.ascontiguousarray(
                np.asarray(prediction[i], dtype=np.float32).reshape(C, HW)),
            "target": np.ascontiguousarray(
                np.asarray(target[i], dtype=np.float32).reshape(C, HW)),
        })
    res = run_bass_kernel_spmd(_get_nc(), in_maps, core_ids=list(range(B)),
                               trace=trace)
    scores = [_score_from_packed(res.results[i]["out"]) for i in range(B)]
    return np.float32(np.mean(scores)), res


def kernel(prediction, target):
    score, _ = run(prediction, target, trace=False)
    return score


# revision 10
# speedup vs baseline: 1.1126x; 1.1126x over previous
"""ArgmaxIOU kernel for 8 Trainium2 NeuronCores.

Data-parallel over batch: core i processes sample i (shapes hardcoded:
B=8, C=21, H=W=512). Raw Bass (explicit engines + semaphores).

Per pixel tile (columns of the per-partition pixel range), per core:
  gpsimd: SWDGE DMA loads with inline f32->bf16 cast (halves SBUF traffic,
          removes the convert stage entirely)
  DVE:    pairwise-max tree over the 21 classes (bf16 2x mode), then
          is_equal against the broadcast max -> one-hot argmax mask,
          scattered into the G-interleaved matmul layout [TB, C, G]
  PE:     matmul eqt^T @ eqp per G-block, accumulating a packed [84, 84]
          confusion matrix in PSUM: conf[c,c'] = sum_g out[4c+g, 4c'+g]

The tile schedule starts and ends with small tiles to shorten pipeline
fill/drain. Host: gather the 8 packed matrices, fold G, compute mean IoU.
bf16 argmax quantization shifts the score by ~5e-4 relative (bf16 argmax
ties are rare and wash out of the large confusion counts).
"""

import sys

import numpy as np

for p in ("/opt/trn_rl_repo",):
    if p not in sys.path:
        sys.path.insert(0, p)

from concourse import bass, mybir
from concourse.bass_utils import run_bass_kernel_spmd

B = 8
C = 21
HW = 512 * 512
P = 128
Q = HW // P              # pixels per partition (2048)
G = 4                    # t-columns packed per matmul (4*21=84 <= 128)
M = G * C                # 84
TMAX = 256
NSLOT = 4                # bf16 data tile slots (DMA runs ahead)
TS = [64, 64, 128] + [256] * 6 + [128, 64, 64]   # sums to 2048
assert sum(TS) == Q
J = len(TS)

F32 = mybir.dt.float32
BF16 = mybir.dt.bfloat16


def build():
    nc = bass.Bass()
    pred = nc.declare_dram_parameter("prediction", [C, HW], F32, isOutput=False)
    targ = nc.declare_dram_parameter("target", [C, HW], F32, isOutput=False)
    out = nc.declare_dram_parameter("out", [M, M], F32, isOutput=True)

    # partition p owns pixels [p*Q, (p+1)*Q); tile j covers columns
    # off_j .. off_j+T_j of every partition
    predv = pred[:].rearrange("c (p q) -> p c q", p=P)
    targv = targ[:].rearrange("c (p q) -> p c q", p=P)

    offs = []
    o = 0
    for t in TS:
        offs.append(o)
        o += t

    mx = mybir.AluOpType.max
    eqop = mybir.AluOpType.is_equal

    with (
        nc.sbuf_tensor("bt", [P, NSLOT, C, TMAX], BF16) as bt,
        nc.sbuf_tensor("bp", [P, NSLOT, C, TMAX], BF16) as bp,
        nc.sbuf_tensor("eqt", [P, 2, TMAX // G, C, G], BF16) as eqt,
        nc.sbuf_tensor("eqp", [P, 2, TMAX // G, C, G], BF16) as eqp,
        nc.sbuf_tensor("st", [P, 10, TMAX], BF16) as st,
        nc.sbuf_tensor("sp", [P, 10, TMAX], BF16) as sp,
        nc.sbuf_tensor("mt", [P, TMAX], BF16) as mt,
        nc.sbuf_tensor("mp", [P, TMAX], BF16) as mp,
        nc.sbuf_tensor("osb", [M, M], F32) as osb,
        nc.psum_tensor("conf", [M, M], F32) as conf,
        nc.semaphore("dml") as dml,
        nc.semaphore("dve") as dve,
        nc.semaphore("mm") as mm,
        nc.semaphore("fin") as fin,
        nc.Block() as block,
    ):

        @block.gpsimd
        def _(g):
            for j in range(J):
                s = j % NSLOT
                off, t = offs[j], TS[j]
                if j >= NSLOT:
                    g.wait_ge(dve, 2 * (j - NSLOT) + 2)  # DVE done with slot
                g.dma_start(out=bt[:, s, :, 0:t],
                            in_=targv[:, :, off:off + t]).then_inc(dml, 16)
                g.dma_start(out=bp[:, s, :, 0:t],
                            in_=predv[:, :, off:off + t]).then_inc(dml, 16)
            g.wait_ge(dve, 2 * J + 1)                    # osb written
            g.dma_start(out=out[:], in_=osb[:]).then_inc(fin, 16)
            g.wait_ge(fin, 16)

        @block.vector
        def _(v):
            def tree(data, scr, mo, t):
                # max over 21 class slices of data [P, C, t] -> mo [P, t]
                v.tensor_tensor(scr[:, 0:10, 0:t], data[:, 0:10, 0:t],
                                data[:, 10:20, 0:t], mx)
                v.tensor_tensor(scr[:, 0:5, 0:t], scr[:, 0:5, 0:t],
                                scr[:, 5:10, 0:t], mx)
                v.tensor_tensor(scr[:, 0:2, 0:t], scr[:, 0:2, 0:t],
                                scr[:, 2:4, 0:t], mx)
                v.tensor_tensor(scr[:, 0:1, 0:t], scr[:, 0:1, 0:t],
                                scr[:, 1:2, 0:t], mx)
                v.tensor_tensor(scr[:, 0:1, 0:t], scr[:, 0:1, 0:t],
                                scr[:, 4:5, 0:t], mx)
                v.tensor_tensor(mo[:, 0:t].unsqueeze(1), scr[:, 0:1, 0:t],
                                data[:, 20:21, 0:t], mx)

            for j in range(J):
                s = j % NSLOT
                e = j % 2
                off, t = offs[j], TS[j]
                tb = t // G
                v.wait_ge(dml, 32 * j + 16)
                tree(bt[:, s], st, mt, t)
                if j >= 2:
                    v.wait_ge(mm, j - 1)                 # PE done with eq slot
                v.tensor_tensor(
                    eqt[:, e, 0:tb].rearrange("p tb c g -> p c tb g"),
                    bt[:, s, :, 0:t].rearrange("p c (tb g) -> p c tb g", g=G),
                    mt[:, 0:t].rearrange("p (tb g) -> p tb g", g=G)
                        .unsqueeze(1).broadcast_to((P, C, tb, G)),
                    eqop).then_inc(dve, 1)
                v.wait_ge(dml, 32 * j + 32)
                tree(bp[:, s], sp, mp, t)
                v.tensor_tensor(
                    eqp[:, e, 0:tb].rearrange("p tb c g -> p c tb g"),
                    bp[:, s, :, 0:t].rearrange("p c (tb g) -> p c tb g", g=G),
                    mp[:, 0:t].rearrange("p (tb g) -> p tb g", g=G)
                        .unsqueeze(1).broadcast_to((P, C, tb, G)),
                    eqop).then_inc(dve, 1)
            v.wait_ge(mm, J)
            v.tensor_copy(osb[:], conf[:]).then_inc(dve, 1)

        @block.tensor
        def _(te):
            for j in range(J):
                e = j % 2
                tbs = TS[j] // G
                te.wait_ge(dve, 2 * j + 2)
                for tb in range(tbs):
                    inst = te.matmul(
                        conf[:],
                        eqt[:, e, tb].rearrange("p c g -> p (c g)"),
                        eqp[:, e, tb].rearrange("p c g -> p (c g)"),
                        start=(j == 0 and tb == 0),
                        stop=(j == J - 1 and tb == tbs - 1))
                    if tb == tbs - 1:
                        inst.then_inc(mm, 1)

    return nc


def _score_from_packed(packed):
    """packed: [84, 84] f32 -> per-sample mean IoU (float64)."""
    x = packed.astype(np.float64).reshape(C, G, C, G)
    conf = np.einsum("igjg->ij", x)
    TP = np.diag(conf).copy()
    FN = conf.sum(axis=1) - TP
    FP = conf.sum(axis=0) - TP
    valid = TP > 0
    denom = TP + FN + FP
    iou = np.where(valid, TP / np.where(valid, denom, 1.0), 0.0)
    n_valid = max(float(valid.sum()), 1.0)
    return iou.sum() / n_valid


_NC_CACHE = {}


def _get_nc():
    if "nc" not in _NC_CACHE:
        _NC_CACHE["nc"] = build()
    return _NC_CACHE["nc"]


def run(prediction, target, trace=False):
    in_maps = []
    for i in range(B):
        in_maps.append({
            "prediction": np.ascontiguousarray(
                np.asarray(prediction[i], dtype=np.float32).reshape(C, HW)),
            "target": np.ascontiguousarray(
                np.asarray(target[i], dtype=np.float32).reshape(C, HW)),
        })
    res = run_bass_kernel_spmd(_get_nc(), in_maps, core_ids=list(range(B)),
                               trace=trace)
    scores = [_score_from_packed(res.results[i]["out"]) for i in range(B)]
    return np.float32(np.mean(scores)), res


def kernel(prediction, target):
    score, _ = run(prediction, target, trace=False)
    return score
